# revision 16
# baseline (speedup 1.0000x reference)
"""Trainium2 Bass kernel for KeypointAlignmentLossL2.

Strategy (data-parallel over batch, one NeuronCore per batch element):
  Host prep (per core b):
    - repack BOTH images into one pair-interleaved pixel-major fp8 tensor
      ("featP"): per image, part A = rows (0,1),(2,3),... interleaved per
      column, part B = rows (1,2),(3,4),... . A keypoint's whole 2x2
      bilinear patch is then 3072 contiguous bytes at a single
      host-computed index (part A for even y0, part B for odd y0, + image
      offset) -> ONE gather descriptor per keypoint (SWDGE desc-gen at
      ~8 ns/desc on the Q7 is the pool-engine bottleneck, and each prep
      costs a fixed ~2us of trigger/IncSwdgeSem overhead, so the kernel
      uses just 2 gather calls of 1024 indices covering both images).
    - bilinear weights packed as 128x128 fp8 diagonal matrices so the lerp
      runs on the tensor engine as accumulating diagonal matmuls
  Device (per core):
    - dma_gather (SWDGE prepare_only + trigger_dma so gathers pipeline
      back-to-back on the DMA engines), keypoint -> partition
    - TensorE: f = sum_nb diag(w_nb) @ g_nb accumulated in PSUM (f32),
      fp8 matmuls; the two N=384 halves land at psum[:, 128:512] and
      [:, 512:896] (each inside one bank, contiguous as a read region).
      Dummy warm-up matmuls keep the PE HAM clock at 2.4 GHz while the
      first gather is in flight.
    - VectorE: copy f2 PSUM->SBUF bf16; scalar_tensor_tensor computes
      dot = sum(f1*f2) in one fused pass
    - ScalarE: activation(Square, accum_out) for |f1|^2 and |f2|^2
    - outputs one [128, 24] f32 tile (n1 | n2 | dot, keypoint-chunk layout)
  Host finish: masked mean of 2 - 2*cos distances across all cores.
"""
import copy as _pycopy
import numpy as np
import ml_dtypes

B, C, H, W, N = 8, 768, 64, 64, 1024
HW_ = H * W
NCHUNK = N // 128   # 8 chunks of 128 keypoints
NPAIR_A = HW_ // 2            # 2048 pair-slots in part A (even y0)
NPAIR_B = (H - 2) // 2 * W    # 1984 pair-slots in part B (odd y0)
NPAIR = NPAIR_A + NPAIR_B     # per image
N_WARM = 90                   # PE warm-up matmuls

_CACHE = {}


def _pair_ap(dram_handle):
    """Flat [2*NPAIR*1536] fp8 dram tensor -> AP [[1536, 2*NPAIR-1],
    [1, 3072]] so dma_gather with elem_step=1536 and elem_size=3072 fetches
    a 2x2 pixel patch per index (idx in pair-slot units, both images)."""
    import bass_rust
    base = dram_handle[:].rearrange("(r c) -> r c", c=3072)
    ap = _pycopy.copy(base)
    ap.ap = bass_rust.VecI64Pair([[1536, 2 * NPAIR - 1], [1, 3072]])
    return ap


def _build_nc():
    from contextlib import ExitStack
    import concourse.bass as bass
    import concourse.tile as tile
    import concourse.mybir as mybir
    from concourse import bacc

    f32 = mybir.dt.float32
    bf16 = mybir.dt.bfloat16
    fp8 = mybir.dt.float8e4
    i16 = mybir.dt.int16
    MULT = mybir.AluOpType.mult
    SQUARE = mybir.ActivationFunctionType.Square

    nc = bacc.Bacc("TRN2", target_bir_lowering=False, debug=False, num_devices=8)

    featP = nc.dram_tensor("featP", [2 * NPAIR * 2 * C], fp8, kind="ExternalInput")
    idx = nc.dram_tensor("idx", [128, 2 * N // 16], i16, kind="ExternalInput")
    wd = nc.dram_tensor("wd", [128, 2 * NCHUNK * 4, 128], fp8, kind="ExternalInput")
    out_res = nc.dram_tensor("out_res", [128, 3 * NCHUNK], f32, kind="ExternalOutput")

    feat_ap = _pair_ap(featP)
    # corner nb (reference order: y0x0, y0x1, y1x0, y1x1) -> byte offset in
    # the gathered pair-interleaved patch [y0x0 | y1x0 | y0x1 | y1x1]
    CORNER_OFF = (0, 2 * C, C, 3 * C)

    with tile.TileContext(nc) as tc, ExitStack() as ctx:
        const_pool = ctx.enter_context(tc.tile_pool(name="const", bufs=1))
        f2c_pool = ctx.enter_context(tc.tile_pool(name="f2c", bufs=3))
        dump_pool = ctx.enter_context(tc.tile_pool(name="dump", bufs=6))
        ppool = ctx.enter_context(
            tc.tile_pool(name="p", bufs=3, space=bass.MemorySpace.PSUM)
        )
        wpool = ctx.enter_context(
            tc.tile_pool(name="warm", bufs=1, space=bass.MemorySpace.PSUM)
        )

        zbias = const_pool.tile([128, 1], f32, tag="zbias", name="zbias")
        nc.vector.memset(zbias[:], 0.0)

        wd_t = const_pool.tile([128, 2 * NCHUNK * 4, 128], fp8, tag="wd")
        nc.sync.dma_start(wd_t[:], wd[:])
        idx_t = const_pool.tile([128, 2 * N // 16], i16, tag="idx", name="idx")
        nc.sync.dma_start(idx_t[:], idx[:])

        # res layout: cols [0:8] = |f1|^2, [8:16] = |f2|^2, [16:24] = dot
        res = const_pool.tile([128, 3 * NCHUNK], f32, tag="res", name="res")

        # 2 gather calls of 1024 idxs: call h covers chunks 4h..4h+3 of both
        # images (ranks 0-3 = image 1, ranks 4-7 = image 2).
        g_tiles = []
        gsems = []
        for half in range(2):
            g = const_pool.tile(
                [128, 8, 4 * C], fp8, tag=f"g{half}", name=f"g{half}"
            )
            g_tiles.append(g)
            sem = nc.alloc_semaphore(f"gsem_{half}")
            gsems.append(sem)
            nc.gpsimd.dma_gather(
                g[:],
                feat_ap,
                idx_t[:, half * 64:(half + 1) * 64],
                1024,
                1024,
                4 * C,
                elem_step=2 * C,
                prepare_only=True,
                sem=sem,
            )
            nc.gpsimd.trigger_dma(count=None)

        # PE warm-up: keep the HAM activity window busy while gathers are in
        # flight so the real matmuls run at 2.4 GHz. Results are discarded.
        warm_ps = wpool.tile([128, 384], f32, tag="warm")
        warm_rhs = wd_t[:, 0:3, :].rearrange("p a b -> p (a b)")
        for _ in range(N_WARM):
            nc.tensor.matmul(warm_ps[:], wd_t[:, 0, :], warm_rhs,
                             start=True, stop=True)

        for ch in range(NCHUNK):
            half, r = ch // 4, ch % 4
            # --- TensorE: bilinear lerp into PSUM, per image ---
            ps = []
            for im in range(2):
                p = ppool.tile([128, 1024], f32, tag="ps")
                for h in range(2):
                    for nb in range(4):
                        k = (im * NCHUNK + ch) * 4 + nb
                        off = CORNER_OFF[nb] + h * 384
                        mm = nc.tensor.matmul(
                            p[:, 128 + 384 * h:512 + 384 * h],
                            wd_t[:, k, :],
                            g_tiles[half][:, 4 * im + r, off:off + 384],
                            start=(nb == 0),
                            stop=(nb == 3),
                        )
                        if nb == 0:
                            # Gate each accumulation group on the gather's
                            # DMA-completion sem; tile's prepare_only path
                            # does not auto-gate on-chip consumers.
                            mm._wait_ge(gsems[half], 16)
                ps.append(p)
            f1_ap = ps[0][:, 128:896]
            f2_ap = ps[1][:, 128:896]

            # --- VectorE: f2 PSUM->SBUF bf16, then fused dot ---
            f2c = f2c_pool.tile([128, 768], bf16, tag="f2c")
            nc.vector.tensor_copy(f2c[:], f2_ap)
            dump_d = dump_pool.tile([128, 768], bf16, tag="dump_d", name="dump_d")
            nc.vector.scalar_tensor_tensor(
                dump_d[:], f1_ap, 1.0, f2c[:], MULT, MULT,
                accum_out=res[:, 16 + ch:16 + ch + 1],
            )

            # --- ScalarE: |f1|^2 and |f2|^2 ---
            dump_a = dump_pool.tile([128, 768], bf16, tag="dump_a", name="dump_a")
            dump_b = dump_pool.tile([128, 768], bf16, tag="dump_b", name="dump_b")
            nc.scalar.activation(
                dump_a[:], f1_ap, SQUARE, bias=zbias[:],
                accum_out=res[:, ch:ch + 1],
            )
            nc.scalar.activation(
                dump_b[:], f2c[:], SQUARE, bias=zbias[:],
                accum_out=res[:, 8 + ch:8 + ch + 1],
            )

        nc.sync.dma_start(out_res[:], res[:])

    nc.compile()
    return nc


def get_nc():
    if "nc" not in _CACHE:
        _CACHE["nc"] = _build_nc()
    return _CACHE["nc"]


def _host_prep_img(feat_b, kp_b):
    """feat_b [C,H,W] f32, kp_b [N,2] f32 ->
    featPair fp8 flat [NPAIR*1536], pidx int32 [N] (pair-slot index of each
    keypoint's 2x2 patch), w f32 [4, N]"""
    fT = np.ascontiguousarray(
        np.asarray(feat_b, np.float32).reshape(C, H, W).transpose(1, 2, 0)
    ).astype(ml_dtypes.float8_e4m3)  # [H, W, C] fp8
    partA = fT.reshape(H // 2, 2, W, C).transpose(0, 2, 1, 3)
    partB = fT[1:H - 1].reshape((H - 2) // 2, 2, W, C).transpose(0, 2, 1, 3)
    featPair = np.concatenate([partA.reshape(-1), partB.reshape(-1)])
    x = np.asarray(kp_b[:, 0], np.float32)
    y = np.asarray(kp_b[:, 1], np.float32)
    x0 = np.minimum(np.floor(x), np.float32(W - 2)).astype(np.float32)
    y0 = np.minimum(np.floor(y), np.float32(H - 2)).astype(np.float32)
    wx = (x - x0).astype(np.float32)
    wy = (y - y0).astype(np.float32)
    x0i = x0.astype(np.int32)
    y0i = y0.astype(np.int32)
    even = (y0i % 2) == 0
    pidx = np.where(
        even,
        (y0i >> 1) * W + x0i,
        NPAIR_A + ((y0i - 1) >> 1) * W + x0i,
    ).astype(np.int32)
    w = np.stack(
        [(1 - wx) * (1 - wy), wx * (1 - wy), (1 - wx) * wy, wx * wy], 0
    ).astype(np.float32)
    return featPair, pidx, w


def _make_idx_layout(pidx1, pidx2):
    """Two [N] pair-slot index arrays -> [128, 2N/16] int16 SBUF layout.
    Sequence order: per gather call h (1024 idxs): [im1 chunks 4h..4h+3,
    im2 chunks 4h..4h+3 (+NPAIR offset)]. Wrapped so sequence element i
    lives at [i%16 (replicated x8), i//16]."""
    p2 = pidx2.astype(np.int32) + NPAIR
    seq = np.concatenate([
        pidx1[0:512], p2[0:512], pidx1[512:1024], p2[512:1024]
    ]).astype(np.int32)
    lay = seq.reshape(-1, 16).T
    return np.tile(lay, (8, 1)).astype(np.int16)


def _make_wd(w1, w2):
    """weights [4,N] f32 per image -> [128, 64, 128] fp8 diagonal matrices"""
    wd = np.zeros((128, 2 * NCHUNK * 4, 128), np.float32)
    r = np.arange(128)
    for im, w in ((0, w1), (1, w2)):
        for ch in range(NCHUNK):
            for nb in range(4):
                k = (im * NCHUNK + ch) * 4 + nb
                wd[r, k, r] = w[nb, ch * 128:(ch + 1) * 128]
    return wd.astype(ml_dtypes.float8_e4m3)


def build_in_maps(feat1, feat2, kp1, kp2):
    in_maps = []
    for b in range(B):
        fP1, pi1, w1 = _host_prep_img(feat1[b], kp1[b])
        fP2, pi2, w2 = _host_prep_img(feat2[b], kp2[b])
        in_maps.append({
            "featP": np.concatenate([fP1, fP2]),
            "idx": _make_idx_layout(pi1, pi2),
            "wd": _make_wd(w1, w2),
        })
    return in_maps


def kernel(feat1, feat2, kp1, kp2, kp1_mask, kp2_mask):
    from concourse.bass_utils import run_bass_kernel_spmd

    feat1 = np.asarray(feat1, np.float32)
    feat2 = np.asarray(feat2, np.float32)
    kp1 = np.asarray(kp1, np.float32)
    kp2 = np.asarray(kp2, np.float32)
    kp1_mask = np.asarray(kp1_mask)
    kp2_mask = np.asarray(kp2_mask)

    nc = get_nc()
    in_maps = build_in_maps(feat1, feat2, kp1, kp2)
    results = run_bass_kernel_spmd(nc, in_maps, list(range(B))).results

    sum_l2 = 0.0
    sum_valid = 0.0
    for b in range(B):
        r = results[b]["out_res"]
        n1sq = r[:, 0:8].T.reshape(-1).astype(np.float64)
        n2sq = r[:, 8:16].T.reshape(-1).astype(np.float64)
        dot = r[:, 16:24].T.reshape(-1).astype(np.float64)
        m1 = np.maximum(np.sqrt(n1sq), 1e-12)
        m2 = np.maximum(np.sqrt(n2sq), 1e-12)
        l2 = n1sq / (m1 * m1) + n2sq / (m2 * m2) - 2.0 * dot / (m1 * m2)
        valid = (kp1_mask[b] & kp2_mask[b]).astype(np.float64)
        sum_l2 += float((l2 * valid).sum())
        sum_valid += float(valid.sum())

    loss = 0.0 if sum_valid == 0 else sum_l2 / max(sum_valid, 1.0)
    return np.float32(loss)


# revision 20
# speedup vs baseline: 1.0330x; 1.0330x over previous
"""Trainium2 Bass kernel for KeypointAlignmentLossL2.

Strategy (data-parallel over batch, one NeuronCore per batch element):
  Host prep (per core b):
    - repack BOTH images into one pair-interleaved pixel-major fp8 tensor
      ("featP"): per image, part A = rows (0,1),(2,3),... interleaved per
      column, part B = rows (1,2),(3,4),... . A keypoint's whole 2x2
      bilinear patch is then 3072 contiguous bytes at a single
      host-computed index (part A for even y0, part B for odd y0, + image
      offset) -> ONE gather descriptor per keypoint (SWDGE desc-gen at
      ~8 ns/desc on the Q7 is the pool-engine bottleneck, and each prep
      costs a fixed ~2us of trigger/IncSwdgeSem overhead, so the kernel
      uses just 2 gather calls of 1024 indices covering both images).
    - bilinear weights packed as 128x128 fp8 diagonal matrices so the lerp
      runs on the tensor engine as accumulating diagonal matmuls
  Device (per core):
    - dma_gather (SWDGE prepare_only + trigger_dma so gathers pipeline
      back-to-back on the DMA engines), keypoint -> partition
    - TensorE: f = sum_nb diag(w_nb) @ g_nb accumulated in PSUM (f32),
      fp8 matmuls; the two N=384 halves land at psum[:, 128:512] and
      [:, 512:896] (each inside one bank, contiguous as a read region).
      Dummy warm-up matmuls keep the PE HAM clock at 2.4 GHz while the
      first gather is in flight.
    - VectorE: copy f2 PSUM->SBUF bf16; scalar_tensor_tensor computes
      dot = sum(f1*f2) in one fused pass
    - ScalarE: activation(Square, accum_out) for |f1|^2 and |f2|^2
    - outputs one [128, 24] f32 tile (n1 | n2 | dot, keypoint-chunk layout)
  Host finish: masked mean of 2 - 2*cos distances across all cores.
"""
import copy as _pycopy
import numpy as np
import ml_dtypes

B, C, H, W, N = 8, 768, 64, 64, 1024
HW_ = H * W
NCHUNK = N // 128   # 8 chunks of 128 keypoints
NPAIR_A = HW_ // 2            # 2048 pair-slots in part A (even y0)
NPAIR_B = (H - 2) // 2 * W    # 1984 pair-slots in part B (odd y0)
NPAIR = NPAIR_A + NPAIR_B     # per image
N_WARM = 90                   # PE warm-up matmuls

_CACHE = {}


def _pair_ap(dram_handle):
    """Flat [2*NPAIR*1536] fp8 dram tensor -> AP [[1536, 2*NPAIR-1],
    [1, 3072]] so dma_gather with elem_step=1536 and elem_size=3072 fetches
    a 2x2 pixel patch per index (idx in pair-slot units, both images)."""
    import bass_rust
    base = dram_handle[:].rearrange("(r c) -> r c", c=3072)
    ap = _pycopy.copy(base)
    ap.ap = bass_rust.VecI64Pair([[1536, 2 * NPAIR - 1], [1, 3072]])
    return ap


def _build_nc():
    from contextlib import ExitStack
    import concourse.bass as bass
    import concourse.tile as tile
    import concourse.mybir as mybir
    from concourse import bacc

    f32 = mybir.dt.float32
    bf16 = mybir.dt.bfloat16
    fp8 = mybir.dt.float8e4
    i16 = mybir.dt.int16
    MULT = mybir.AluOpType.mult
    SQUARE = mybir.ActivationFunctionType.Square

    nc = bacc.Bacc("TRN2", target_bir_lowering=False, debug=False, num_devices=8)

    featP = nc.dram_tensor("featP", [2 * NPAIR * 2 * C], fp8, kind="ExternalInput")
    idx = nc.dram_tensor("idx", [128, 2 * N // 16], i16, kind="ExternalInput")
    wd = nc.dram_tensor("wd", [128, 2 * NCHUNK * 4, 128], fp8, kind="ExternalInput")
    out_res = nc.dram_tensor("out_res", [128, 3 * NCHUNK], f32, kind="ExternalOutput")

    feat_ap = _pair_ap(featP)
    # corner nb (reference order: y0x0, y0x1, y1x0, y1x1) -> byte offset in
    # the gathered pair-interleaved patch [y0x0 | y1x0 | y0x1 | y1x1]
    CORNER_OFF = (0, 2 * C, C, 3 * C)

    with tile.TileContext(nc) as tc, ExitStack() as ctx:
        const_pool = ctx.enter_context(tc.tile_pool(name="const", bufs=1))
        f2c_pool = ctx.enter_context(tc.tile_pool(name="f2c", bufs=3))
        dump_pool = ctx.enter_context(tc.tile_pool(name="dump", bufs=6))
        ppool = ctx.enter_context(
            tc.tile_pool(name="p", bufs=4, space=bass.MemorySpace.PSUM)
        )

        zbias = const_pool.tile([128, 1], f32, tag="zbias", name="zbias")
        nc.vector.memset(zbias[:], 0.0)

        wd_t = const_pool.tile([128, 2 * NCHUNK * 4, 128], fp8, tag="wd")
        nc.sync.dma_start(wd_t[:], wd[:])
        idx_t = const_pool.tile([128, 2 * N // 16], i16, tag="idx", name="idx")
        nc.sync.dma_start(idx_t[:], idx[:])

        # res layout: cols [0:8] = |f1|^2, [8:16] = |f2|^2, [16:24] = dot
        res = const_pool.tile([128, 3 * NCHUNK], f32, tag="res", name="res")

        # 3 gather calls covering both images: chunks 0-3 (1024 idxs), then
        # 4-5 and 6-7 (512 idxs each) so the final transfer - which gates the
        # compute tail - is short.
        CALLS = ((0, 4), (4, 6), (6, 8))  # chunk ranges
        g_tiles = []
        gsems = []
        call_of_chunk = {}
        for ci, (c0, c1) in enumerate(CALLS):
            nch = c1 - c0
            g = const_pool.tile(
                [128, 2 * nch, 4 * C], fp8, tag=f"g{ci}", name=f"g{ci}"
            )
            g_tiles.append(g)
            sem = nc.alloc_semaphore(f"gsem_{ci}")
            gsems.append(sem)
            nc.gpsimd.dma_gather(
                g[:],
                feat_ap,
                idx_t[:, c0 * 16:c1 * 16],
                nch * 256,
                nch * 256,
                4 * C,
                elem_step=2 * C,
                prepare_only=True,
                sem=sem,
            )
            nc.gpsimd.trigger_dma(count=None)
            for ch in range(c0, c1):
                call_of_chunk[ch] = (ci, ch - c0, nch)

        # PE warm-up: keep the HAM activity window busy while gathers are in
        # flight so the real matmuls run at 2.4 GHz. Results are discarded.
        warm_ps = ppool.tile([128, 1024], f32, tag="ps")
        warm_rhs = wd_t[:, 0:3, :].rearrange("p a b -> p (a b)")
        for _ in range(N_WARM):
            nc.tensor.matmul(warm_ps[:, 128:512], wd_t[:, 0, :], warm_rhs,
                             start=True, stop=True)

        for ch in range(NCHUNK):
            ci, r, nch = call_of_chunk[ch]
            # --- TensorE: bilinear lerp into PSUM, per image ---
            ps = []
            for im in range(2):
                p = ppool.tile([128, 1024], f32, tag="ps")
                for h in range(2):
                    for nb in range(4):
                        k = (im * NCHUNK + ch) * 4 + nb
                        off = CORNER_OFF[nb] + h * 384
                        mm = nc.tensor.matmul(
                            p[:, 128 + 384 * h:512 + 384 * h],
                            wd_t[:, k, :],
                            g_tiles[ci][:, nch * im + r, off:off + 384],
                            start=(nb == 0),
                            stop=(nb == 3),
                        )
                        if nb == 0:
                            # Gate each accumulation group on the gather's
                            # DMA-completion sem; tile's prepare_only path
                            # does not auto-gate on-chip consumers.
                            mm._wait_ge(gsems[ci], 16)
                ps.append(p)
            f1_ap = ps[0][:, 128:896]
            f2_ap = ps[1][:, 128:896]

            # --- VectorE: f2 PSUM->SBUF bf16, then fused dot ---
            f2c = f2c_pool.tile([128, 768], bf16, tag="f2c")
            nc.vector.tensor_copy(f2c[:], f2_ap)
            dump_d = dump_pool.tile([128, 768], bf16, tag="dump_d", name="dump_d")
            nc.vector.scalar_tensor_tensor(
                dump_d[:], f1_ap, 1.0, f2c[:], MULT, MULT,
                accum_out=res[:, 16 + ch:16 + ch + 1],
            )

            # --- ScalarE: |f1|^2 and |f2|^2 ---
            dump_a = dump_pool.tile([128, 768], bf16, tag="dump_a", name="dump_a")
            dump_b = dump_pool.tile([128, 768], bf16, tag="dump_b", name="dump_b")
            nc.scalar.activation(
                dump_a[:], f1_ap, SQUARE, bias=zbias[:],
                accum_out=res[:, ch:ch + 1],
            )
            nc.scalar.activation(
                dump_b[:], f2c[:], SQUARE, bias=zbias[:],
                accum_out=res[:, 8 + ch:8 + ch + 1],
            )

        nc.sync.dma_start(out_res[:], res[:])

    nc.compile()
    return nc


def get_nc():
    if "nc" not in _CACHE:
        _CACHE["nc"] = _build_nc()
    return _CACHE["nc"]


def _host_prep_img(feat_b, kp_b):
    """feat_b [C,H,W] f32, kp_b [N,2] f32 ->
    featPair fp8 flat [NPAIR*1536], pidx int32 [N] (pair-slot index of each
    keypoint's 2x2 patch), w f32 [4, N]"""
    fT = np.ascontiguousarray(
        np.asarray(feat_b, np.float32).reshape(C, H, W).transpose(1, 2, 0)
    ).astype(ml_dtypes.float8_e4m3)  # [H, W, C] fp8
    partA = fT.reshape(H // 2, 2, W, C).transpose(0, 2, 1, 3)
    partB = fT[1:H - 1].reshape((H - 2) // 2, 2, W, C).transpose(0, 2, 1, 3)
    featPair = np.concatenate([partA.reshape(-1), partB.reshape(-1)])
    x = np.asarray(kp_b[:, 0], np.float32)
    y = np.asarray(kp_b[:, 1], np.float32)
    x0 = np.minimum(np.floor(x), np.float32(W - 2)).astype(np.float32)
    y0 = np.minimum(np.floor(y), np.float32(H - 2)).astype(np.float32)
    wx = (x - x0).astype(np.float32)
    wy = (y - y0).astype(np.float32)
    x0i = x0.astype(np.int32)
    y0i = y0.astype(np.int32)
    even = (y0i % 2) == 0
    pidx = np.where(
        even,
        (y0i >> 1) * W + x0i,
        NPAIR_A + ((y0i - 1) >> 1) * W + x0i,
    ).astype(np.int32)
    w = np.stack(
        [(1 - wx) * (1 - wy), wx * (1 - wy), (1 - wx) * wy, wx * wy], 0
    ).astype(np.float32)
    return featPair, pidx, w


def _make_idx_layout(pidx1, pidx2):
    """Two [N] pair-slot index arrays -> [128, 2N/16] int16 SBUF layout.
    Sequence order: per gather call (chunk ranges 0-4, 4-6, 6-8): [im1
    chunks, im2 chunks (+NPAIR offset)]. Wrapped so sequence element i
    lives at [i%16 (replicated x8), i//16]."""
    p2 = pidx2.astype(np.int32) + NPAIR
    seq = np.concatenate([
        pidx1[0:512], p2[0:512],
        pidx1[512:768], p2[512:768],
        pidx1[768:1024], p2[768:1024],
    ]).astype(np.int32)
    lay = seq.reshape(-1, 16).T
    return np.tile(lay, (8, 1)).astype(np.int16)


def _make_wd(w1, w2):
    """weights [4,N] f32 per image -> [128, 64, 128] fp8 diagonal matrices"""
    wd = np.zeros((128, 2 * NCHUNK * 4, 128), np.float32)
    r = np.arange(128)
    for im, w in ((0, w1), (1, w2)):
        for ch in range(NCHUNK):
            for nb in range(4):
                k = (im * NCHUNK + ch) * 4 + nb
                wd[r, k, r] = w[nb, ch * 128:(ch + 1) * 128]
    return wd.astype(ml_dtypes.float8_e4m3)


def build_in_maps(feat1, feat2, kp1, kp2):
    in_maps = []
    for b in range(B):
        fP1, pi1, w1 = _host_prep_img(feat1[b], kp1[b])
        fP2, pi2, w2 = _host_prep_img(feat2[b], kp2[b])
        in_maps.append({
            "featP": np.concatenate([fP1, fP2]),
            "idx": _make_idx_layout(pi1, pi2),
            "wd": _make_wd(w1, w2),
        })
    return in_maps


def kernel(feat1, feat2, kp1, kp2, kp1_mask, kp2_mask):
    from concourse.bass_utils import run_bass_kernel_spmd

    feat1 = np.asarray(feat1, np.float32)
    feat2 = np.asarray(feat2, np.float32)
    kp1 = np.asarray(kp1, np.float32)
    kp2 = np.asarray(kp2, np.float32)
    kp1_mask = np.asarray(kp1_mask)
    kp2_mask = np.asarray(kp2_mask)

    nc = get_nc()
    in_maps = build_in_maps(feat1, feat2, kp1, kp2)
    results = run_bass_kernel_spmd(nc, in_maps, list(range(B))).results

    sum_l2 = 0.0
    sum_valid = 0.0
    for b in range(B):
        r = results[b]["out_res"]
        n1sq = r[:, 0:8].T.reshape(-1).astype(np.float64)
        n2sq = r[:, 8:16].T.reshape(-1).astype(np.float64)
        dot = r[:, 16:24].T.reshape(-1).astype(np.float64)
        m1 = np.maximum(np.sqrt(n1sq), 1e-12)
        m2 = np.maximum(np.sqrt(n2sq), 1e-12)
        l2 = n1sq / (m1 * m1) + n2sq / (m2 * m2) - 2.0 * dot / (m1 * m2)
        valid = (kp1_mask[b] & kp2_mask[b]).astype(np.float64)
        sum_l2 += float((l2 * valid).sum())
        sum_valid += float(valid.sum())

    loss = 0.0 if sum_valid == 0 else sum_l2 / max(sum_valid, 1.0)
    return np.float32(loss)


# revision 23
# speedup vs baseline: 1.1319x; 1.0958x over previous
"""Trainium2 Bass kernel for KeypointAlignmentLossL2.

Strategy (data-parallel over batch, one NeuronCore per batch element):
  Host prep (per core b):
    - repack BOTH images into one pair-interleaved pixel-major fp8 tensor
      ("featP"): per image, part A = rows (0,1),(2,3),... interleaved per
      column, part B = rows (1,2),(3,4),... . A keypoint's whole 2x2
      bilinear patch is then 3072 contiguous bytes at a single
      host-computed index (part A for even y0, part B for odd y0, + image
      offset) -> ONE gather descriptor per keypoint (SWDGE desc-gen at
      ~8 ns/desc on the Q7 is the pool-engine bottleneck, and each prep
      costs a fixed ~2us of trigger/IncSwdgeSem overhead, so the kernel
      uses just 2 gather calls of 1024 indices covering both images).
    - bilinear weights packed as 128x128 fp8 diagonal matrices so the lerp
      runs on the tensor engine as accumulating diagonal matmuls
  Device (per core):
    - dma_gather (SWDGE prepare_only + trigger_dma so gathers pipeline
      back-to-back on the DMA engines), keypoint -> partition
    - TensorE: f = sum_nb diag(w_nb) @ g_nb accumulated in PSUM (f32),
      fp8 matmuls; the two N=384 halves land at psum[:, 128:512] and
      [:, 512:896] (each inside one bank, contiguous as a read region).
      Dummy warm-up matmuls keep the PE HAM clock at 2.4 GHz while the
      first gather is in flight.
    - VectorE: copy f2 PSUM->SBUF bf16; scalar_tensor_tensor computes
      dot = sum(f1*f2) in one fused pass
    - ScalarE: activation(Square, accum_out) for |f1|^2 and |f2|^2
    - outputs one [128, 24] f32 tile (n1 | n2 | dot, keypoint-chunk layout)
  Host finish: masked mean of 2 - 2*cos distances across all cores.
"""
import copy as _pycopy
import numpy as np
import ml_dtypes

B, C, H, W, N = 8, 768, 64, 64, 1024
HW_ = H * W
NCHUNK = N // 128   # 8 chunks of 128 keypoints
NPAIR_A = HW_ // 2            # 2048 pair-slots in part A (even y0)
NPAIR_B = (H - 2) // 2 * W    # 1984 pair-slots in part B (odd y0)
NPAIR = NPAIR_A + NPAIR_B     # per image
N_WARM = 90                   # PE warm-up matmuls

_CACHE = {}


def _pair_ap(dram_handle):
    """Flat [2*NPAIR*1536] fp8 dram tensor -> AP [[1536, 2*NPAIR-1],
    [1, 3072]] so dma_gather with elem_step=1536 and elem_size=3072 fetches
    a 2x2 pixel patch per index (idx in pair-slot units, both images)."""
    import bass_rust
    base = dram_handle[:].rearrange("(r c) -> r c", c=3072)
    ap = _pycopy.copy(base)
    ap.ap = bass_rust.VecI64Pair([[1536, 2 * NPAIR - 1], [1, 3072]])
    return ap


def _build_nc():
    from contextlib import ExitStack
    import concourse.bass as bass
    import concourse.tile as tile
    import concourse.mybir as mybir
    from concourse import bacc

    f32 = mybir.dt.float32
    bf16 = mybir.dt.bfloat16
    fp8 = mybir.dt.float8e4
    i16 = mybir.dt.int16
    MULT = mybir.AluOpType.mult
    SQUARE = mybir.ActivationFunctionType.Square

    nc = bacc.Bacc("TRN2", target_bir_lowering=False, debug=False, num_devices=8)

    featP = nc.dram_tensor("featP", [2 * NPAIR * 2 * C], fp8, kind="ExternalInput")
    idx = nc.dram_tensor("idx", [128, 2 * N // 16], i16, kind="ExternalInput")
    wd = nc.dram_tensor("wd", [128, 2 * NCHUNK * 4, 128], fp8, kind="ExternalInput")
    out_res = nc.dram_tensor("out_res", [128, 3 * NCHUNK], f32, kind="ExternalOutput")

    feat_ap = _pair_ap(featP)
    # corner nb (reference order: y0x0, y0x1, y1x0, y1x1) -> byte offset in
    # the gathered pair-interleaved patch [y0x0 | y1x0 | y0x1 | y1x1]
    CORNER_OFF = (0, 2 * C, C, 3 * C)

    from concourse import library_config

    with tile.TileContext(nc) as tc, ExitStack() as ctx:
        # Load the GPSIMD library containing dma_gather immediately: the
        # ~10us Q7 library DMA then overlaps the HWDGE input loads instead
        # of delaying the first gather prep.
        nc.gpsimd.load_library(library_config.mlp)

        const_pool = ctx.enter_context(tc.tile_pool(name="const", bufs=1))
        f2c_pool = ctx.enter_context(tc.tile_pool(name="f2c", bufs=3))
        dump_pool = ctx.enter_context(tc.tile_pool(name="dump", bufs=6))
        ppool = ctx.enter_context(
            tc.tile_pool(name="p", bufs=4, space=bass.MemorySpace.PSUM)
        )

        zbias = const_pool.tile([128, 1], f32, tag="zbias", name="zbias")
        nc.vector.memset(zbias[:], 0.0)

        wd_t = const_pool.tile([128, 2 * NCHUNK * 4, 128], fp8, tag="wd")
        nc.sync.dma_start(wd_t[:], wd[:])
        idx_t = const_pool.tile([128, 2 * N // 16], i16, tag="idx", name="idx")
        nc.sync.dma_start(idx_t[:], idx[:])

        # res layout: cols [0:8] = |f1|^2, [8:16] = |f2|^2, [16:24] = dot
        res = const_pool.tile([128, 3 * NCHUNK], f32, tag="res", name="res")

        # 3 gather calls covering both images: chunks 0-3 (1024 idxs), then
        # 4-5 and 6-7 (512 idxs each) so the final transfer - which gates the
        # compute tail - is short.
        CALLS = ((0, 4), (4, 6), (6, 8))  # chunk ranges
        g_tiles = []
        gsems = []
        call_of_chunk = {}
        for ci, (c0, c1) in enumerate(CALLS):
            nch = c1 - c0
            g = const_pool.tile(
                [128, 2 * nch, 4 * C], fp8, tag=f"g{ci}", name=f"g{ci}"
            )
            g_tiles.append(g)
            sem = nc.alloc_semaphore(f"gsem_{ci}")
            gsems.append(sem)
            nc.gpsimd.dma_gather(
                g[:],
                feat_ap,
                idx_t[:, c0 * 16:c1 * 16],
                nch * 256,
                nch * 256,
                4 * C,
                elem_step=2 * C,
                prepare_only=True,
                sem=sem,
            )
            nc.gpsimd.trigger_dma(count=None)
            for ch in range(c0, c1):
                call_of_chunk[ch] = (ci, ch - c0, nch)

        # PE warm-up: keep the HAM activity window busy while gathers are in
        # flight so the real matmuls run at 2.4 GHz. Results are discarded.
        warm_ps = ppool.tile([128, 1024], f32, tag="ps")
        warm_rhs = wd_t[:, 0:3, :].rearrange("p a b -> p (a b)")
        for _ in range(N_WARM):
            nc.tensor.matmul(warm_ps[:, 128:512], wd_t[:, 0, :], warm_rhs,
                             start=True, stop=True)

        DR = mybir.MatmulPerfMode.DoubleRow
        for ch in range(NCHUNK):
            ci, r, nch = call_of_chunk[ch]
            # --- TensorE: bilinear lerp into PSUM, per image ---
            # DoubleRow fp8: each matmul contracts TWO corners (K=256 via the
            # 2-fp8-per-cell interleave), halving PE streaming time. Corner
            # pairs (y0x0,y1x0) and (y0x1,y1x1) are C-strided adjacent blocks
            # in the gathered patch; wd packs the matching diag pairs at
            # consecutive k-slots (see _make_wd).
            ps = []
            for im in range(2):
                p = ppool.tile([128, 1024], f32, tag="ps")
                g3 = g_tiles[ci][:, nch * im + r, :].rearrange(
                    "p (a b) -> p a b", a=4
                )  # [128, 4, 768]: a = corner block (y0x0, y1x0, y0x1, y1x1)
                for h in range(2):
                    for t in range(2):
                        kk = (im * NCHUNK + ch) * 4 + 2 * t
                        mm = nc.tensor.matmul(
                            p[:, 128 + 384 * h:512 + 384 * h],
                            wd_t[:, kk:kk + 2, :],
                            g3[:, 2 * t:2 * t + 2, h * 384:h * 384 + 384],
                            start=(t == 0),
                            stop=(t == 1),
                            perf_mode=DR,
                        )
                        if t == 0:
                            # Gate each accumulation group on the gather's
                            # DMA-completion sem; tile's prepare_only path
                            # does not auto-gate on-chip consumers.
                            mm._wait_ge(gsems[ci], 16)
                ps.append(p)
            f1_ap = ps[0][:, 128:896]
            f2_ap = ps[1][:, 128:896]

            # --- VectorE: f2 PSUM->SBUF bf16, then fused dot ---
            f2c = f2c_pool.tile([128, 768], bf16, tag="f2c")
            nc.vector.tensor_copy(f2c[:], f2_ap)
            dump_d = dump_pool.tile([128, 768], bf16, tag="dump_d", name="dump_d")
            nc.vector.scalar_tensor_tensor(
                dump_d[:], f1_ap, 1.0, f2c[:], MULT, MULT,
                accum_out=res[:, 16 + ch:16 + ch + 1],
            )

            # --- ScalarE: |f1|^2 and |f2|^2 ---
            dump_a = dump_pool.tile([128, 768], bf16, tag="dump_a", name="dump_a")
            dump_b = dump_pool.tile([128, 768], bf16, tag="dump_b", name="dump_b")
            nc.scalar.activation(
                dump_a[:], f1_ap, SQUARE, bias=zbias[:],
                accum_out=res[:, ch:ch + 1],
            )
            nc.scalar.activation(
                dump_b[:], f2c[:], SQUARE, bias=zbias[:],
                accum_out=res[:, 8 + ch:8 + ch + 1],
            )

        nc.sync.dma_start(out_res[:], res[:])

    nc.compile()
    return nc


def get_nc():
    if "nc" not in _CACHE:
        _CACHE["nc"] = _build_nc()
    return _CACHE["nc"]


def _host_prep_img(feat_b, kp_b):
    """feat_b [C,H,W] f32, kp_b [N,2] f32 ->
    featPair fp8 flat [NPAIR*1536], pidx int32 [N] (pair-slot index of each
    keypoint's 2x2 patch), w f32 [4, N]"""
    fT = np.ascontiguousarray(
        np.asarray(feat_b, np.float32).reshape(C, H, W).transpose(1, 2, 0)
    ).astype(ml_dtypes.float8_e4m3)  # [H, W, C] fp8
    partA = fT.reshape(H // 2, 2, W, C).transpose(0, 2, 1, 3)
    partB = fT[1:H - 1].reshape((H - 2) // 2, 2, W, C).transpose(0, 2, 1, 3)
    featPair = np.concatenate([partA.reshape(-1), partB.reshape(-1)])
    x = np.asarray(kp_b[:, 0], np.float32)
    y = np.asarray(kp_b[:, 1], np.float32)
    x0 = np.minimum(np.floor(x), np.float32(W - 2)).astype(np.float32)
    y0 = np.minimum(np.floor(y), np.float32(H - 2)).astype(np.float32)
    wx = (x - x0).astype(np.float32)
    wy = (y - y0).astype(np.float32)
    x0i = x0.astype(np.int32)
    y0i = y0.astype(np.int32)
    even = (y0i % 2) == 0
    pidx = np.where(
        even,
        (y0i >> 1) * W + x0i,
        NPAIR_A + ((y0i - 1) >> 1) * W + x0i,
    ).astype(np.int32)
    w = np.stack(
        [(1 - wx) * (1 - wy), wx * (1 - wy), (1 - wx) * wy, wx * wy], 0
    ).astype(np.float32)
    return featPair, pidx, w


def _make_idx_layout(pidx1, pidx2):
    """Two [N] pair-slot index arrays -> [128, 2N/16] int16 SBUF layout.
    Sequence order: per gather call (chunk ranges 0-4, 4-6, 6-8): [im1
    chunks, im2 chunks (+NPAIR offset)]. Wrapped so sequence element i
    lives at [i%16 (replicated x8), i//16]."""
    p2 = pidx2.astype(np.int32) + NPAIR
    seq = np.concatenate([
        pidx1[0:512], p2[0:512],
        pidx1[512:768], p2[512:768],
        pidx1[768:1024], p2[768:1024],
    ]).astype(np.int32)
    lay = seq.reshape(-1, 16).T
    return np.tile(lay, (8, 1)).astype(np.int16)


def _make_wd(w1, w2):
    """weights [4,N] f32 per image -> [128, 64, 128] fp8 diagonal matrices.
    k-slot order per (im, chunk): [w0, w2, w1, w3] so DoubleRow corner
    pairs (y0x0,y1x0) and (y0x1,y1x1) sit at consecutive slots."""
    wd = np.zeros((128, 2 * NCHUNK * 4, 128), np.float32)
    r = np.arange(128)
    SLOT = (0, 2, 1, 3)  # reference nb -> k-slot within the chunk
    for im, w in ((0, w1), (1, w2)):
        for ch in range(NCHUNK):
            for nb in range(4):
                k = (im * NCHUNK + ch) * 4 + SLOT[nb]
                wd[r, k, r] = w[nb, ch * 128:(ch + 1) * 128]
    return wd.astype(ml_dtypes.float8_e4m3)


def build_in_maps(feat1, feat2, kp1, kp2):
    in_maps = []
    for b in range(B):
        fP1, pi1, w1 = _host_prep_img(feat1[b], kp1[b])
        fP2, pi2, w2 = _host_prep_img(feat2[b], kp2[b])
        in_maps.append({
            "featP": np.concatenate([fP1, fP2]),
            "idx": _make_idx_layout(pi1, pi2),
            "wd": _make_wd(w1, w2),
        })
    return in_maps


def kernel(feat1, feat2, kp1, kp2, kp1_mask, kp2_mask):
    from concourse.bass_utils import run_bass_kernel_spmd

    feat1 = np.asarray(feat1, np.float32)
    feat2 = np.asarray(feat2, np.float32)
    kp1 = np.asarray(kp1, np.float32)
    kp2 = np.asarray(kp2, np.float32)
    kp1_mask = np.asarray(kp1_mask)
    kp2_mask = np.asarray(kp2_mask)

    nc = get_nc()
    in_maps = build_in_maps(feat1, feat2, kp1, kp2)
    results = run_bass_kernel_spmd(nc, in_maps, list(range(B))).results

    sum_l2 = 0.0
    sum_valid = 0.0
    for b in range(B):
        r = results[b]["out_res"]
        n1sq = r[:, 0:8].T.reshape(-1).astype(np.float64)
        n2sq = r[:, 8:16].T.reshape(-1).astype(np.float64)
        dot = r[:, 16:24].T.reshape(-1).astype(np.float64)
        m1 = np.maximum(np.sqrt(n1sq), 1e-12)
        m2 = np.maximum(np.sqrt(n2sq), 1e-12)
        l2 = n1sq / (m1 * m1) + n2sq / (m2 * m2) - 2.0 * dot / (m1 * m2)
        valid = (kp1_mask[b] & kp2_mask[b]).astype(np.float64)
        sum_l2 += float((l2 * valid).sum())
        sum_valid += float(valid.sum())

    loss = 0.0 if sum_valid == 0 else sum_l2 / max(sum_valid, 1.0)
    return np.float32(loss)


# revision 25
# speedup vs baseline: 1.1910x; 1.0522x over previous
"""Trainium2 Bass kernel for KeypointAlignmentLossL2.

Strategy (data-parallel over batch, one NeuronCore per batch element):
  Host prep (per core b):
    - repack BOTH images into one pair-interleaved pixel-major fp8 tensor
      ("featP"): per image, part A = rows (0,1),(2,3),... interleaved per
      column, part B = rows (1,2),(3,4),... . A keypoint's whole 2x2
      bilinear patch is then 3072 contiguous bytes at a single
      host-computed index (part A for even y0, part B for odd y0, + image
      offset) -> ONE gather descriptor per keypoint (SWDGE desc-gen at
      ~8 ns/desc on the Q7 is the pool-engine bottleneck, and each prep
      costs a fixed ~2us of trigger/IncSwdgeSem overhead, so the kernel
      uses just 2 gather calls of 1024 indices covering both images).
    - bilinear weights packed as 128x128 fp8 diagonal matrices so the lerp
      runs on the tensor engine as accumulating diagonal matmuls
  Device (per core):
    - dma_gather (SWDGE prepare_only + trigger_dma so gathers pipeline
      back-to-back on the DMA engines), keypoint -> partition
    - TensorE: f = sum_nb diag(w_nb) @ g_nb accumulated in PSUM (f32),
      fp8 matmuls; the two N=384 halves land at psum[:, 128:512] and
      [:, 512:896] (each inside one bank, contiguous as a read region).
      Dummy warm-up matmuls keep the PE HAM clock at 2.4 GHz while the
      first gather is in flight.
    - VectorE: copy f2 PSUM->SBUF bf16; scalar_tensor_tensor computes
      dot = sum(f1*f2) in one fused pass
    - ScalarE: activation(Square, accum_out) for |f1|^2 and |f2|^2
    - outputs one [128, 24] f32 tile (n1 | n2 | dot, keypoint-chunk layout)
  Host finish: masked mean of 2 - 2*cos distances across all cores.
"""
import copy as _pycopy
import numpy as np
import ml_dtypes

B, C, H, W, N = 8, 768, 64, 64, 1024
HW_ = H * W
NCHUNK = N // 128   # 8 chunks of 128 keypoints
NPAIR_A = HW_ // 2            # 2048 pair-slots in part A (even y0)
NPAIR_B = (H - 2) // 2 * W    # 1984 pair-slots in part B (odd y0)
NPAIR = NPAIR_A + NPAIR_B     # per image
N_WARM = 90                   # PE warm-up matmuls

_CACHE = {}


def _pair_ap(dram_handle):
    """Flat [2*NPAIR*1536] fp8 dram tensor -> AP [[1536, 2*NPAIR-1],
    [1, 3072]] so dma_gather with elem_step=1536 and elem_size=3072 fetches
    a 2x2 pixel patch per index (idx in pair-slot units, both images)."""
    import bass_rust
    base = dram_handle[:].rearrange("(r c) -> r c", c=3072)
    ap = _pycopy.copy(base)
    ap.ap = bass_rust.VecI64Pair([[1536, 2 * NPAIR - 1], [1, 3072]])
    return ap


def _build_nc():
    from contextlib import ExitStack
    import concourse.bass as bass
    import concourse.tile as tile
    import concourse.mybir as mybir
    from concourse import bacc

    f32 = mybir.dt.float32
    bf16 = mybir.dt.bfloat16
    fp8 = mybir.dt.float8e4
    i16 = mybir.dt.int16
    MULT = mybir.AluOpType.mult
    SQUARE = mybir.ActivationFunctionType.Square

    nc = bacc.Bacc("TRN2", target_bir_lowering=False, debug=False, num_devices=8)

    featP = nc.dram_tensor("featP", [2 * NPAIR * 2 * C], fp8, kind="ExternalInput")
    idx = nc.dram_tensor("idx", [128, 2 * N // 16], i16, kind="ExternalInput")
    wd = nc.dram_tensor("wd", [128, 2 * NCHUNK * 4, 128], fp8, kind="ExternalInput")
    out_res = nc.dram_tensor("out_res", [128, 3 * NCHUNK], f32, kind="ExternalOutput")

    feat_ap = _pair_ap(featP)
    # corner nb (reference order: y0x0, y0x1, y1x0, y1x1) -> byte offset in
    # the gathered pair-interleaved patch [y0x0 | y1x0 | y0x1 | y1x1]
    CORNER_OFF = (0, 2 * C, C, 3 * C)

    from concourse import library_config

    with tile.TileContext(nc) as tc, ExitStack() as ctx:
        # Load the GPSIMD library containing dma_gather immediately: the
        # ~10us Q7 library DMA then overlaps the HWDGE input loads instead
        # of delaying the first gather prep.
        nc.gpsimd.load_library(library_config.mlp)

        const_pool = ctx.enter_context(tc.tile_pool(name="const", bufs=1))
        f2c_pool = ctx.enter_context(tc.tile_pool(name="f2c", bufs=3))
        dump_pool = ctx.enter_context(tc.tile_pool(name="dump", bufs=6))
        ppool = ctx.enter_context(
            tc.tile_pool(name="p", bufs=4, space=bass.MemorySpace.PSUM)
        )

        zbias = const_pool.tile([128, 1], f32, tag="zbias", name="zbias")
        nc.vector.memset(zbias[:], 0.0)

        wd_t = const_pool.tile([128, 2 * NCHUNK * 4, 128], fp8, tag="wd")
        nc.sync.dma_start(wd_t[:], wd[:])
        idx_t = const_pool.tile([128, 2 * N // 16], i16, tag="idx", name="idx")
        nc.sync.dma_start(idx_t[:], idx[:])

        # res layout: cols [0:8] = |f1|^2, [8:16] = |f2|^2, [16:24] = dot
        res = const_pool.tile([128, 3 * NCHUNK], f32, tag="res", name="res")

        # 4 gather calls covering both images, 2 chunks each: the first
        # transfer (which gates compute start) fires after only ~4.5us of
        # desc-gen, and data then arrives incrementally so the DVE/ACT
        # reduction pipeline is never starved then swamped.
        CALLS = ((0, 2), (2, 4), (4, 6), (6, 8))  # chunk ranges
        g_tiles = []
        gsems = []
        call_of_chunk = {}
        for ci, (c0, c1) in enumerate(CALLS):
            nch = c1 - c0
            g = const_pool.tile(
                [128, 2 * nch, 4 * C], fp8, tag=f"g{ci}", name=f"g{ci}"
            )
            g_tiles.append(g)
            sem = nc.alloc_semaphore(f"gsem_{ci}")
            gsems.append(sem)
            nc.gpsimd.dma_gather(
                g[:],
                feat_ap,
                idx_t[:, c0 * 16:c1 * 16],
                nch * 256,
                nch * 256,
                4 * C,
                elem_step=2 * C,
                prepare_only=True,
                sem=sem,
            )
            nc.gpsimd.trigger_dma(count=None)
            for ch in range(c0, c1):
                call_of_chunk[ch] = (ci, ch - c0, nch)

        # PE warm-up: keep the HAM activity window busy while gathers are in
        # flight so the real matmuls run at 2.4 GHz. Results are discarded.
        warm_ps = ppool.tile([128, 1024], f32, tag="ps")
        warm_rhs = wd_t[:, 0:3, :].rearrange("p a b -> p (a b)")
        for _ in range(N_WARM):
            nc.tensor.matmul(warm_ps[:, 128:512], wd_t[:, 0, :], warm_rhs,
                             start=True, stop=True)

        DR = mybir.MatmulPerfMode.DoubleRow
        for ch in range(NCHUNK):
            ci, r, nch = call_of_chunk[ch]
            # --- TensorE: bilinear lerp into PSUM, per image ---
            # DoubleRow fp8: each matmul contracts TWO corners (K=256 via the
            # 2-fp8-per-cell interleave), halving PE streaming time. Corner
            # pairs (y0x0,y1x0) and (y0x1,y1x1) are C-strided adjacent blocks
            # in the gathered patch; wd packs the matching diag pairs at
            # consecutive k-slots (see _make_wd).
            ps = []
            for im in range(2):
                p = ppool.tile([128, 1024], f32, tag="ps")
                g3 = g_tiles[ci][:, nch * im + r, :].rearrange(
                    "p (a b) -> p a b", a=4
                )  # [128, 4, 768]: a = corner block (y0x0, y1x0, y0x1, y1x1)
                for h in range(2):
                    for t in range(2):
                        kk = (im * NCHUNK + ch) * 4 + 2 * t
                        mm = nc.tensor.matmul(
                            p[:, 128 + 384 * h:512 + 384 * h],
                            wd_t[:, kk:kk + 2, :],
                            g3[:, 2 * t:2 * t + 2, h * 384:h * 384 + 384],
                            start=(t == 0),
                            stop=(t == 1),
                            perf_mode=DR,
                        )
                        if t == 0:
                            # Gate each accumulation group on the gather's
                            # DMA-completion sem; tile's prepare_only path
                            # does not auto-gate on-chip consumers.
                            mm._wait_ge(gsems[ci], 16)
                ps.append(p)
            f1_ap = ps[0][:, 128:896]
            f2_ap = ps[1][:, 128:896]

            # --- VectorE: f2 PSUM->SBUF bf16, then fused dot ---
            f2c = f2c_pool.tile([128, 768], bf16, tag="f2c")
            nc.vector.tensor_copy(f2c[:], f2_ap)
            dump_d = dump_pool.tile([128, 768], bf16, tag="dump_d", name="dump_d")
            nc.vector.scalar_tensor_tensor(
                dump_d[:], f1_ap, 1.0, f2c[:], MULT, MULT,
                accum_out=res[:, 16 + ch:16 + ch + 1],
            )

            # --- ScalarE: |f1|^2 and |f2|^2 ---
            dump_a = dump_pool.tile([128, 768], bf16, tag="dump_a", name="dump_a")
            dump_b = dump_pool.tile([128, 768], bf16, tag="dump_b", name="dump_b")
            nc.scalar.activation(
                dump_a[:], f1_ap, SQUARE, bias=zbias[:],
                accum_out=res[:, ch:ch + 1],
            )
            nc.scalar.activation(
                dump_b[:], f2c[:], SQUARE, bias=zbias[:],
                accum_out=res[:, 8 + ch:8 + ch + 1],
            )

        nc.sync.dma_start(out_res[:], res[:])

    nc.compile()
    return nc


def get_nc():
    if "nc" not in _CACHE:
        _CACHE["nc"] = _build_nc()
    return _CACHE["nc"]


def _host_prep_img(feat_b, kp_b):
    """feat_b [C,H,W] f32, kp_b [N,2] f32 ->
    featPair fp8 flat [NPAIR*1536], pidx int32 [N] (pair-slot index of each
    keypoint's 2x2 patch), w f32 [4, N]"""
    fT = np.ascontiguousarray(
        np.asarray(feat_b, np.float32).reshape(C, H, W).transpose(1, 2, 0)
    ).astype(ml_dtypes.float8_e4m3)  # [H, W, C] fp8
    partA = fT.reshape(H // 2, 2, W, C).transpose(0, 2, 1, 3)
    partB = fT[1:H - 1].reshape((H - 2) // 2, 2, W, C).transpose(0, 2, 1, 3)
    featPair = np.concatenate([partA.reshape(-1), partB.reshape(-1)])
    x = np.asarray(kp_b[:, 0], np.float32)
    y = np.asarray(kp_b[:, 1], np.float32)
    x0 = np.minimum(np.floor(x), np.float32(W - 2)).astype(np.float32)
    y0 = np.minimum(np.floor(y), np.float32(H - 2)).astype(np.float32)
    wx = (x - x0).astype(np.float32)
    wy = (y - y0).astype(np.float32)
    x0i = x0.astype(np.int32)
    y0i = y0.astype(np.int32)
    even = (y0i % 2) == 0
    pidx = np.where(
        even,
        (y0i >> 1) * W + x0i,
        NPAIR_A + ((y0i - 1) >> 1) * W + x0i,
    ).astype(np.int32)
    w = np.stack(
        [(1 - wx) * (1 - wy), wx * (1 - wy), (1 - wx) * wy, wx * wy], 0
    ).astype(np.float32)
    return featPair, pidx, w


def _make_idx_layout(pidx1, pidx2):
    """Two [N] pair-slot index arrays -> [128, 2N/16] int16 SBUF layout.
    Sequence order: per gather call (2 chunks each): [im1 chunks, im2
    chunks (+NPAIR offset)]. Wrapped so sequence element i lives at
    [i%16 (replicated x8), i//16]."""
    p2 = pidx2.astype(np.int32) + NPAIR
    seq = np.concatenate([
        np.concatenate([pidx1[g * 256:(g + 1) * 256], p2[g * 256:(g + 1) * 256]])
        for g in range(4)
    ]).astype(np.int32)
    lay = seq.reshape(-1, 16).T
    return np.tile(lay, (8, 1)).astype(np.int16)


def _make_wd(w1, w2):
    """weights [4,N] f32 per image -> [128, 64, 128] fp8 diagonal matrices.
    k-slot order per (im, chunk): [w0, w2, w1, w3] so DoubleRow corner
    pairs (y0x0,y1x0) and (y0x1,y1x1) sit at consecutive slots."""
    wd = np.zeros((128, 2 * NCHUNK * 4, 128), np.float32)
    r = np.arange(128)
    SLOT = (0, 2, 1, 3)  # reference nb -> k-slot within the chunk
    for im, w in ((0, w1), (1, w2)):
        for ch in range(NCHUNK):
            for nb in range(4):
                k = (im * NCHUNK + ch) * 4 + SLOT[nb]
                wd[r, k, r] = w[nb, ch * 128:(ch + 1) * 128]
    return wd.astype(ml_dtypes.float8_e4m3)


def build_in_maps(feat1, feat2, kp1, kp2):
    in_maps = []
    for b in range(B):
        fP1, pi1, w1 = _host_prep_img(feat1[b], kp1[b])
        fP2, pi2, w2 = _host_prep_img(feat2[b], kp2[b])
        in_maps.append({
            "featP": np.concatenate([fP1, fP2]),
            "idx": _make_idx_layout(pi1, pi2),
            "wd": _make_wd(w1, w2),
        })
    return in_maps


def kernel(feat1, feat2, kp1, kp2, kp1_mask, kp2_mask):
    from concourse.bass_utils import run_bass_kernel_spmd

    feat1 = np.asarray(feat1, np.float32)
    feat2 = np.asarray(feat2, np.float32)
    kp1 = np.asarray(kp1, np.float32)
    kp2 = np.asarray(kp2, np.float32)
    kp1_mask = np.asarray(kp1_mask)
    kp2_mask = np.asarray(kp2_mask)

    nc = get_nc()
    in_maps = build_in_maps(feat1, feat2, kp1, kp2)
    results = run_bass_kernel_spmd(nc, in_maps, list(range(B))).results

    sum_l2 = 0.0
    sum_valid = 0.0
    for b in range(B):
        r = results[b]["out_res"]
        n1sq = r[:, 0:8].T.reshape(-1).astype(np.float64)
        n2sq = r[:, 8:16].T.reshape(-1).astype(np.float64)
        dot = r[:, 16:24].T.reshape(-1).astype(np.float64)
        m1 = np.maximum(np.sqrt(n1sq), 1e-12)
        m2 = np.maximum(np.sqrt(n2sq), 1e-12)
        l2 = n1sq / (m1 * m1) + n2sq / (m2 * m2) - 2.0 * dot / (m1 * m2)
        valid = (kp1_mask[b] & kp2_mask[b]).astype(np.float64)
        sum_l2 += float((l2 * valid).sum())
        sum_valid += float(valid.sum())

    loss = 0.0 if sum_valid == 0 else sum_l2 / max(sum_valid, 1.0)
    return np.float32(loss)


# revision 27
# speedup vs baseline: 1.2020x; 1.0092x over previous
"""Trainium2 Bass kernel for KeypointAlignmentLossL2.

Strategy (data-parallel over batch, one NeuronCore per batch element):
  Host prep (per core b):
    - repack BOTH images into one pair-interleaved pixel-major fp8 tensor
      ("featP"): per image, part A = rows (0,1),(2,3),... interleaved per
      column, part B = rows (1,2),(3,4),... . A keypoint's whole 2x2
      bilinear patch is then 3072 contiguous bytes at a single
      host-computed index (part A for even y0, part B for odd y0, + image
      offset) -> ONE gather descriptor per keypoint (SWDGE desc-gen at
      ~8 ns/desc on the Q7 is the pool-engine bottleneck, and each prep
      costs a fixed ~2us of trigger/IncSwdgeSem overhead, so the kernel
      uses just 2 gather calls of 1024 indices covering both images).
    - bilinear weights packed as 128x128 fp8 diagonal matrices so the lerp
      runs on the tensor engine as accumulating diagonal matmuls
  Device (per core):
    - dma_gather (SWDGE prepare_only + trigger_dma so gathers pipeline
      back-to-back on the DMA engines), keypoint -> partition
    - TensorE: f = sum_nb diag(w_nb) @ g_nb accumulated in PSUM (f32),
      fp8 matmuls; the two N=384 halves land at psum[:, 128:512] and
      [:, 512:896] (each inside one bank, contiguous as a read region).
      Dummy warm-up matmuls keep the PE HAM clock at 2.4 GHz while the
      first gather is in flight.
    - VectorE: copy f2 PSUM->SBUF bf16; scalar_tensor_tensor computes
      dot = sum(f1*f2) in one fused pass
    - ScalarE: activation(Square, accum_out) for |f1|^2 and |f2|^2
    - outputs one [128, 24] f32 tile (n1 | n2 | dot, keypoint-chunk layout)
  Host finish: masked mean of 2 - 2*cos distances across all cores.
"""
import copy as _pycopy
import numpy as np
import ml_dtypes

B, C, H, W, N = 8, 768, 64, 64, 1024
HW_ = H * W
NCHUNK = N // 128   # 8 chunks of 128 keypoints
NPAIR_A = HW_ // 2            # 2048 pair-slots in part A (even y0)
NPAIR_B = (H - 2) // 2 * W    # 1984 pair-slots in part B (odd y0)
NPAIR = NPAIR_A + NPAIR_B     # per image
N_WARM = 90                   # PE warm-up matmuls

_CACHE = {}


def _pair_ap(dram_handle):
    """Flat [2*NPAIR*1536] fp8 dram tensor -> AP [[1536, 2*NPAIR-1],
    [1, 3072]] so dma_gather with elem_step=1536 and elem_size=3072 fetches
    a 2x2 pixel patch per index (idx in pair-slot units, both images)."""
    import bass_rust
    base = dram_handle[:].rearrange("(r c) -> r c", c=3072)
    ap = _pycopy.copy(base)
    ap.ap = bass_rust.VecI64Pair([[1536, 2 * NPAIR - 1], [1, 3072]])
    return ap


def _build_nc():
    from contextlib import ExitStack
    import concourse.bass as bass
    import concourse.tile as tile
    import concourse.mybir as mybir
    from concourse import bacc

    f32 = mybir.dt.float32
    bf16 = mybir.dt.bfloat16
    fp8 = mybir.dt.float8e4
    i16 = mybir.dt.int16
    MULT = mybir.AluOpType.mult
    SQUARE = mybir.ActivationFunctionType.Square

    nc = bacc.Bacc("TRN2", target_bir_lowering=False, debug=False, num_devices=8)

    featP = nc.dram_tensor("featP", [2 * NPAIR * 2 * C], fp8, kind="ExternalInput")
    idx = nc.dram_tensor("idx", [128, 2 * N // 16], i16, kind="ExternalInput")
    wd = nc.dram_tensor("wd", [128, 2 * NCHUNK * 4, 128], fp8, kind="ExternalInput")
    out_res = nc.dram_tensor("out_res", [128, 3 * NCHUNK], f32, kind="ExternalOutput")

    feat_ap = _pair_ap(featP)
    # corner nb (reference order: y0x0, y0x1, y1x0, y1x1) -> byte offset in
    # the gathered pair-interleaved patch [y0x0 | y1x0 | y0x1 | y1x1]
    CORNER_OFF = (0, 2 * C, C, 3 * C)

    from concourse import library_config

    with tile.TileContext(nc) as tc, ExitStack() as ctx:
        # Load the GPSIMD library containing dma_gather immediately: the
        # ~10us Q7 library DMA then overlaps the HWDGE input loads instead
        # of delaying the first gather prep.
        nc.gpsimd.load_library(library_config.mlp)

        const_pool = ctx.enter_context(tc.tile_pool(name="const", bufs=1))
        f2c_pool = ctx.enter_context(tc.tile_pool(name="f2c", bufs=3))
        dump_pool = ctx.enter_context(tc.tile_pool(name="dump", bufs=6))
        ppool = ctx.enter_context(
            tc.tile_pool(name="p", bufs=4, space=bass.MemorySpace.PSUM)
        )

        zbias = const_pool.tile([128, 1], f32, tag="zbias", name="zbias")
        nc.vector.memset(zbias[:], 0.0)

        wd_t = const_pool.tile([128, 2 * NCHUNK * 4, 128], fp8, tag="wd")
        nc.sync.dma_start(wd_t[:], wd[:])
        idx_t = const_pool.tile([128, 2 * N // 16], i16, tag="idx", name="idx")
        nc.sync.dma_start(idx_t[:], idx[:])

        # res layout: cols [0:8] = |f1|^2, [8:16] = |f2|^2, [16:24] = dot
        res = const_pool.tile([128, 3 * NCHUNK], f32, tag="res", name="res")

        # 3 gather calls covering both images (3+3+2 chunks): each prep costs
        # gen + a fixed ~1.9us trigger/IncSwdgeSem tax on the serial Pool
        # chain, so fewer calls pull the LAST trigger (which gates the final
        # transfer and the compute tail) earlier; the short 2-chunk final
        # call keeps the tail transfer small.
        CALLS = ((0, 3), (3, 6), (6, 8))  # chunk ranges
        g_tiles = []
        gsems = []
        call_of_chunk = {}
        for ci, (c0, c1) in enumerate(CALLS):
            nch = c1 - c0
            g = const_pool.tile(
                [128, 2 * nch, 4 * C], fp8, tag=f"g{ci}", name=f"g{ci}"
            )
            g_tiles.append(g)
            sem = nc.alloc_semaphore(f"gsem_{ci}")
            gsems.append(sem)
            nc.gpsimd.dma_gather(
                g[:],
                feat_ap,
                idx_t[:, c0 * 16:c1 * 16],
                nch * 256,
                nch * 256,
                4 * C,
                elem_step=2 * C,
                prepare_only=True,
                sem=sem,
            )
            nc.gpsimd.trigger_dma(count=None)
            for ch in range(c0, c1):
                call_of_chunk[ch] = (ci, ch - c0, nch)

        # PE warm-up: keep the HAM activity window busy while gathers are in
        # flight so the real matmuls run at 2.4 GHz. Results are discarded.
        warm_ps = ppool.tile([128, 1024], f32, tag="ps")
        warm_rhs = wd_t[:, 0:3, :].rearrange("p a b -> p (a b)")
        for _ in range(N_WARM):
            nc.tensor.matmul(warm_ps[:, 128:512], wd_t[:, 0, :], warm_rhs,
                             start=True, stop=True)

        DR = mybir.MatmulPerfMode.DoubleRow
        for ch in range(NCHUNK):
            ci, r, nch = call_of_chunk[ch]
            # --- TensorE: bilinear lerp into PSUM, per image ---
            # DoubleRow fp8: each matmul contracts TWO corners (K=256 via the
            # 2-fp8-per-cell interleave), halving PE streaming time. Corner
            # pairs (y0x0,y1x0) and (y0x1,y1x1) are C-strided adjacent blocks
            # in the gathered patch; wd packs the matching diag pairs at
            # consecutive k-slots (see _make_wd).
            ps = []
            for im in range(2):
                p = ppool.tile([128, 1024], f32, tag="ps")
                g3 = g_tiles[ci][:, nch * im + r, :].rearrange(
                    "p (a b) -> p a b", a=4
                )  # [128, 4, 768]: a = corner block (y0x0, y1x0, y0x1, y1x1)
                for h in range(2):
                    for t in range(2):
                        kk = (im * NCHUNK + ch) * 4 + 2 * t
                        mm = nc.tensor.matmul(
                            p[:, 128 + 384 * h:512 + 384 * h],
                            wd_t[:, kk:kk + 2, :],
                            g3[:, 2 * t:2 * t + 2, h * 384:h * 384 + 384],
                            start=(t == 0),
                            stop=(t == 1),
                            perf_mode=DR,
                        )
                        if t == 0:
                            # Gate each accumulation group on the gather's
                            # DMA-completion sem; tile's prepare_only path
                            # does not auto-gate on-chip consumers.
                            mm._wait_ge(gsems[ci], 16)
                ps.append(p)
            f1_ap = ps[0][:, 128:896]
            f2_ap = ps[1][:, 128:896]

            # --- VectorE: f2 PSUM->SBUF bf16, then fused dot ---
            f2c = f2c_pool.tile([128, 768], bf16, tag="f2c")
            nc.vector.tensor_copy(f2c[:], f2_ap)
            dump_d = dump_pool.tile([128, 768], bf16, tag="dump_d", name="dump_d")
            nc.vector.scalar_tensor_tensor(
                dump_d[:], f1_ap, 1.0, f2c[:], MULT, MULT,
                accum_out=res[:, 16 + ch:16 + ch + 1],
            )

            # --- ScalarE: |f1|^2 and |f2|^2 ---
            dump_a = dump_pool.tile([128, 768], bf16, tag="dump_a", name="dump_a")
            dump_b = dump_pool.tile([128, 768], bf16, tag="dump_b", name="dump_b")
            nc.scalar.activation(
                dump_a[:], f1_ap, SQUARE, bias=zbias[:],
                accum_out=res[:, ch:ch + 1],
            )
            nc.scalar.activation(
                dump_b[:], f2c[:], SQUARE, bias=zbias[:],
                accum_out=res[:, 8 + ch:8 + ch + 1],
            )

        nc.sync.dma_start(out_res[:], res[:])

    nc.compile()
    return nc


def get_nc():
    if "nc" not in _CACHE:
        _CACHE["nc"] = _build_nc()
    return _CACHE["nc"]


def _host_prep_img(feat_b, kp_b):
    """feat_b [C,H,W] f32, kp_b [N,2] f32 ->
    featPair fp8 flat [NPAIR*1536], pidx int32 [N] (pair-slot index of each
    keypoint's 2x2 patch), w f32 [4, N]"""
    fT = np.ascontiguousarray(
        np.asarray(feat_b, np.float32).reshape(C, H, W).transpose(1, 2, 0)
    ).astype(ml_dtypes.float8_e4m3)  # [H, W, C] fp8
    partA = fT.reshape(H // 2, 2, W, C).transpose(0, 2, 1, 3)
    partB = fT[1:H - 1].reshape((H - 2) // 2, 2, W, C).transpose(0, 2, 1, 3)
    featPair = np.concatenate([partA.reshape(-1), partB.reshape(-1)])
    x = np.asarray(kp_b[:, 0], np.float32)
    y = np.asarray(kp_b[:, 1], np.float32)
    x0 = np.minimum(np.floor(x), np.float32(W - 2)).astype(np.float32)
    y0 = np.minimum(np.floor(y), np.float32(H - 2)).astype(np.float32)
    wx = (x - x0).astype(np.float32)
    wy = (y - y0).astype(np.float32)
    x0i = x0.astype(np.int32)
    y0i = y0.astype(np.int32)
    even = (y0i % 2) == 0
    pidx = np.where(
        even,
        (y0i >> 1) * W + x0i,
        NPAIR_A + ((y0i - 1) >> 1) * W + x0i,
    ).astype(np.int32)
    w = np.stack(
        [(1 - wx) * (1 - wy), wx * (1 - wy), (1 - wx) * wy, wx * wy], 0
    ).astype(np.float32)
    return featPair, pidx, w


def _make_idx_layout(pidx1, pidx2):
    """Two [N] pair-slot index arrays -> [128, 2N/16] int16 SBUF layout.
    Sequence order: per gather call (2 chunks each): [im1 chunks, im2
    chunks (+NPAIR offset)]. Wrapped so sequence element i lives at
    [i%16 (replicated x8), i//16]."""
    p2 = pidx2.astype(np.int32) + NPAIR
    seq = np.concatenate([
        np.concatenate([pidx1[a * 128:b * 128], p2[a * 128:b * 128]])
        for a, b in ((0, 3), (3, 6), (6, 8))
    ]).astype(np.int32)
    lay = seq.reshape(-1, 16).T
    return np.tile(lay, (8, 1)).astype(np.int16)


def _make_wd(w1, w2):
    """weights [4,N] f32 per image -> [128, 64, 128] fp8 diagonal matrices.
    k-slot order per (im, chunk): [w0, w2, w1, w3] so DoubleRow corner
    pairs (y0x0,y1x0) and (y0x1,y1x1) sit at consecutive slots."""
    wd = np.zeros((128, 2 * NCHUNK * 4, 128), np.float32)
    r = np.arange(128)
    SLOT = (0, 2, 1, 3)  # reference nb -> k-slot within the chunk
    for im, w in ((0, w1), (1, w2)):
        for ch in range(NCHUNK):
            for nb in range(4):
                k = (im * NCHUNK + ch) * 4 + SLOT[nb]
                wd[r, k, r] = w[nb, ch * 128:(ch + 1) * 128]
    return wd.astype(ml_dtypes.float8_e4m3)


def build_in_maps(feat1, feat2, kp1, kp2):
    in_maps = []
    for b in range(B):
        fP1, pi1, w1 = _host_prep_img(feat1[b], kp1[b])
        fP2, pi2, w2 = _host_prep_img(feat2[b], kp2[b])
        in_maps.append({
            "featP": np.concatenate([fP1, fP2]),
            "idx": _make_idx_layout(pi1, pi2),
            "wd": _make_wd(w1, w2),
        })
    return in_maps


def kernel(feat1, feat2, kp1, kp2, kp1_mask, kp2_mask):
    from concourse.bass_utils import run_bass_kernel_spmd

    feat1 = np.asarray(feat1, np.float32)
    feat2 = np.asarray(feat2, np.float32)
    kp1 = np.asarray(kp1, np.float32)
    kp2 = np.asarray(kp2, np.float32)
    kp1_mask = np.asarray(kp1_mask)
    kp2_mask = np.asarray(kp2_mask)

    nc = get_nc()
    in_maps = build_in_maps(feat1, feat2, kp1, kp2)
    results = run_bass_kernel_spmd(nc, in_maps, list(range(B))).results

    sum_l2 = 0.0
    sum_valid = 0.0
    for b in range(B):
        r = results[b]["out_res"]
        n1sq = r[:, 0:8].T.reshape(-1).astype(np.float64)
        n2sq = r[:, 8:16].T.reshape(-1).astype(np.float64)
        dot = r[:, 16:24].T.reshape(-1).astype(np.float64)
        m1 = np.maximum(np.sqrt(n1sq), 1e-12)
        m2 = np.maximum(np.sqrt(n2sq), 1e-12)
        l2 = n1sq / (m1 * m1) + n2sq / (m2 * m2) - 2.0 * dot / (m1 * m2)
        valid = (kp1_mask[b] & kp2_mask[b]).astype(np.float64)
        sum_l2 += float((l2 * valid).sum())
        sum_valid += float(valid.sum())

    loss = 0.0 if sum_valid == 0 else sum_l2 / max(sum_valid, 1.0)
    return np.float32(loss)
